# revision 18
# baseline (speedup 1.0000x reference)
"""Trainium2 Bass kernel for a GPT-2-style transformer block (B=2, T=2048, C=768).

Sharding: 8 cores = 2 batch rows x 4 sequence-group cores. Each core handles
512 query tokens chosen as q-tiles {g, 7-g, 8+g, 15-g} of its batch row (this
balances causal-attention work exactly across the 4 cores of a batch group).
Each core redundantly computes LN1 + K/V projections over its full context
(no cross-core communication). All matmuls run in bf16 with fp32 PSUM
accumulation; LayerNorm statistics, softmax accumulation and residuals stay
in fp32. LN gains are folded into the following weight matrices on the host;
all weight matrices are pre-swizzled on the host into the SBUF-resident
layout so every weight DMA is a single contiguous transfer.

Schedule strategy (v1):
- Two HWDGE queues: activations (xq, x chunks) on sync/SP, weights+masks on
  scalar/ACT in need-order, so phase 1 starts immediately.
- LayerNorm stats run chunk-batched on DVE (bn_stats) with a single ACT
  Rsqrt per 4-tile chunk; normalization runs on DVE (tensor_scalar with two
  per-partition scalars). ACT is kept free for softmax exp, which is the
  attention-interval bottleneck (~1 elem/cycle/partition).
- Token->feature-major transposes run on the DMA xbar (dma_start_transpose),
  freeing the PE entirely for matmuls.
- Phase 2 emits Q first, then K/V per 512-token chunk; attention is
  restructured chunk-outer/head-inner with softmax/AV accumulation in SBUF
  (y_acc), so exp work for early k-chunks overlaps the remaining projection
  matmuls.
- The two 64-row score matmuls (head-pair halves) are issued back-to-back
  with tile_position=(0,0)/(64,0) so they run concurrently in different PE
  row-groups.
"""
import os
import sys
from contextlib import ExitStack

for _p in ("/opt/trn_rl_repo", "/root/.axon_site/_ro/trn_rl_repo"):
    if os.path.isdir(_p) and _p not in sys.path:
        sys.path.insert(0, _p)

import numpy as np
import ml_dtypes

import concourse.bass as bass
import concourse.tile as tile
from concourse import mybir
from concourse.bass_utils import run_bass_kernel_spmd
from concourse.vector_clock import ScopedClock

# ---------------------------------------------------------------- dimensions
B, T, C = 2, 2048, 768
H, HD = 12, 64
DFF = 4 * C
EPS = 1e-5
P = 128
NT = T // P            # 16 k/q tiles per batch row
GQ = 4                 # q-tiles per core
TOK = GQ * P           # 512 query tokens per core
NC6 = C // P           # 6
ND = DFF // P          # 24
QTILES = [sorted([g, 7 - g, 8 + g, 15 - g]) for g in range(4)]

dt = mybir.dt
F32, BF16 = dt.float32, dt.bfloat16

# ------------------------------------------------- drain sem-wait splitting
# The neuronxcc walrus in this environment rejects instructions carrying more
# than a few semaphore waits; the Tile kernel-tail drain can exceed that.
# Split the drain's waits across a chain of drains, one wait each.
_MAXW = 1


def _patched_drain_and_barrier(self, tick_clock, wait_clock):
    nc_ = self.nc
    probe = nc_.sync.drain()
    wait_clock.add_sem_waits(probe.ins, ScopedClock({None: tick_clock.global_clock}))
    si = probe.ins.sync_info
    waits = list(si.on_wait or []) if si is not None else []
    if len(waits) > _MAXW:
        probe.ins.sync_info.on_wait = waits[:_MAXW]
        rest = waits[_MAXW:]
        while rest:
            extra = nc_.sync.drain()
            extra.ins.sync_info = mybir.SyncInfo(on_wait=rest[:_MAXW], on_update=[])
            rest = rest[_MAXW:]
    nc_.all_engine_barrier()
    popped = nc_._tile_sem_poison_stack.pop()
    assert popped is self._sem_poison
    nc_.clear_and_free_semaphores(list(self.sems.allocated().values()))
    nc_.all_engine_barrier()


tile.TileContext._drain_and_barrier = _patched_drain_and_barrier


SPLIT_WAITS = True
DMA_TRANSPOSE = True
MASK_BCAST = True
BUFS_PSS = 2
BUFS_PSY = 2
BUFS_PSA = 2
BUFS_PEXP = 3
BUFS_WORK = 3
MAX_WAITS = 1


def _split_excess_waits(nc, max_waits: int | None = None):
    if max_waits is None:
        max_waits = MAX_WAITS
    """This environment's walrus rejects instructions with more than a couple
    of semaphore waits. Hoist excess waits onto same-engine no-ops inserted
    directly before the over-subscribed instruction."""
    n_split = 0
    for f in nc.m.functions:
        for bb in f.blocks:
            new_insts = []
            for inst in bb.instructions:
                si = inst.sync_info
                waits = list(si.on_wait) if (si is not None and si.on_wait) else []
                if len(waits) > max_waits:
                    rest = waits[:-max_waits]
                    inst.sync_info.on_wait = waits[-max_waits:]
                    k = 0
                    while rest:
                        nop = mybir.InstNoOp(
                            name=f"{inst.name}-wsplit{k}", ins=[], outs=[])
                        nop.engine = inst.engine
                        nop.sync_info = mybir.SyncInfo(
                            on_wait=rest[:max_waits], on_update=[])
                        new_insts.append(nop)
                        rest = rest[max_waits:]
                        k += 1
                    n_split += 1
                new_insts.append(inst)
            bb.instructions = new_insts
    return n_split


# ------------------------------------------------------------ program build
def build_program(nreps: int = 1, timing: bool = False) -> bass.Bass:
    nc = bass.Bass()
    AF = mybir.ActivationFunctionType
    OP = mybir.AluOpType

    if timing:
        # Timing variant: identical instruction stream, but all big tensors
        # are kernel-internal DRAM (uninitialized garbage - timing is
        # data-independent) so repeated executions don't pay per-call host
        # input copies. Tiny dummy I/O keeps the PJRT plumbing happy.
        def din(name, shape, dtp):
            return nc.dram_tensor(name, shape, dtp)
        tick_d = nc.declare_dram_parameter("tick", [1, 1], F32, isOutput=False)
        tock_d = nc.declare_dram_parameter("tock", [1, 1], F32, isOutput=True)
    else:
        def din(name, shape, dtp):
            return nc.declare_dram_parameter(name, shape, dtp, isOutput=False)

    x_ctx_d = din("x_ctx", [T, C], F32)
    xq_d = din("xq", [TOK, C], F32)
    ident_d = din("ident", [P, P], BF16)
    ones_d = din("ones_row", [1, 64], dt.float32r)
    wqkv_d = din("w_qkv", [P, 3 * NC6 * C], BF16)
    qkvb_d = din("qkv_b", [P, 3 * NC6], F32)
    vb_d = din("vb_row", [1, C], F32)
    wo_d = din("w_o", [P, NC6 * C], BF16)
    wfc_d = din("w_fc", [P, NC6 * DFF], BF16)
    fcb_d = din("fc_b", [P, ND], F32)
    wproj_d = din("w_proj", [P, ND * C], BF16)
    bproj_d = din("bproj_row", [1, C], F32)
    masks_d = din("masks", [P, NT * P], BF16)
    if timing:
        out_d = nc.dram_tensor("out_q", [TOK, C], F32)
    else:
        out_d = nc.declare_dram_parameter("out_q", [TOK, C], F32, isOutput=True)

    with tile.TileContext(nc) as tc:
        if timing:
            with tc.tile_pool(name="tickp", bufs=1) as tickp:
                tick_t = tickp.tile([1, 1], F32, name="tick_t")
                nc.sync.dma_start(out=tick_t, in_=tick_d[:, :])
                nc.sync.dma_start(out=tock_d[:, :], in_=tick_t)
        def body():
            emit_block(nc, tc, AF, OP,
                       x_ctx_d, xq_d, ident_d, ones_d, wqkv_d, qkvb_d, vb_d,
                       wo_d, wfc_d, fcb_d, wproj_d, bproj_d, masks_d, out_d)

        for _ in range(nreps):
            body()
    if SPLIT_WAITS:
        _split_excess_waits(nc)
    return nc


def emit_block(nc, tc, AF, OP, x_ctx_d, xq_d, ident_d, ones_d, wqkv_d, qkvb_d,
               vb_d, wo_d, wfc_d, fcb_d, wproj_d, bproj_d, masks_d, out_d):
    F32R = dt.float32r
    with ExitStack() as es:
        constp = es.enter_context(tc.tile_pool(name="constp", bufs=1))
        work = es.enter_context(tc.tile_pool(name="work", bufs=BUFS_WORK))
        small = es.enter_context(tc.tile_pool(name="small", bufs=4))
        pC = es.enter_context(tc.tile_pool(name="pC", bufs=1))

        # ---- constants
        eps_t = constp.tile([P, 1], F32, name="eps_t")
        nc.vector.memset(eps_t, EPS)
        ones64 = constp.tile([1, 64], F32R, name="ones64")
        nc.gpsimd.dma_start(out=ones64, in_=ones_d[:, :])
        qkvb = constp.tile([P, 3 * NC6], F32, name="qkvb")
        nc.gpsimd.dma_start(out=qkvb, in_=qkvb_d[:, :])
        fcb = constp.tile([P, ND], F32, name="fcb")
        nc.gpsimd.dma_start(out=fcb, in_=fcb_d[:, :])
        vb_bc = constp.tile([P, C], F32, name="vb_bc")
        nc.gpsimd.dma_start(out=vb_bc, in_=vb_d[:, :].to_broadcast((P, C)))
        bproj_bc = constp.tile([P, C], F32, name="bproj_bc")
        nc.gpsimd.dma_start(out=bproj_bc, in_=bproj_d[:, :].to_broadcast((P, C)))
        if not DMA_TRANSPOSE:
            ident = constp.tile([P, P], BF16, name="ident")
            nc.gpsimd.dma_start(out=ident, in_=ident_d[:, :])

        # ---- long-lived activations (whole-kernel scope)
        xq_sb = pC.tile([P, GQ, C], F32, name="xq_sb")
        y_fm = pC.tile([P, NC6, TOK], BF16, name="y_fm")
        wo_sb = pC.tile([P, NC6, C], BF16, name="wo_sb")

        def ln_chunk(xc_ap, nt, emit_tile, act_norm=False):
            """Chunk-batched LayerNorm: stats on DVE; rstd computed as
            exp(-0.5*ln(var+eps)) so ACT stays inside the ln/exp table set
            (a Sqrt would force a ~2.7us table switch against softmax exp).
            Normalization on DVE (or ACT when act_norm and ACT is idle).
            emit_tile(t, lt) is called with the normalized bf16 tile."""
            st = small.tile([P, nt, 2, 6], F32, name="bn_st")
            mv = small.tile([P, nt, 2], F32, name="bn_mv")
            for t in range(nt):
                for sg in range(2):
                    nc.vector.bn_stats(out=st[:, t, sg, :],
                                       in_=xc_ap[:, t, sg * 384:(sg + 1) * 384])
                nc.vector.bn_aggr(out=mv[:, t, :], in_=st[:, t])
            rs = small.tile([P, nt], F32, name="bn_rs")
            nc.scalar.activation(out=rs, in_=mv[:, :, 1], func=AF.Ln,
                                 bias=eps_t)
            nc.scalar.activation(out=rs, in_=rs, func=AF.Exp, scale=-0.5)
            nb = small.tile([P, nt], F32, name="bn_nb")
            nc.vector.scalar_tensor_tensor(out=nb, in0=mv[:, :, 0], scalar=-1.0,
                                           in1=rs, op0=OP.mult, op1=OP.mult)
            for t in range(nt):
                lt = work.tile([P, C], BF16, name="ln_lt")
                if act_norm:
                    nc.scalar.activation(out=lt, in_=xc_ap[:, t, :],
                                         func=AF.Identity,
                                         bias=nb[:, t:t + 1],
                                         scale=rs[:, t:t + 1])
                else:
                    nc.vector.tensor_scalar(out=lt, in0=xc_ap[:, t, :],
                                            scalar1=rs[:, t:t + 1],
                                            scalar2=nb[:, t:t + 1],
                                            op0=OP.mult, op1=OP.add)
                emit_tile(t, lt)

        def transpose_fm(dst, t0, lt, ps_pool, dma_eng):
            """Transpose bf16 [P, C] token-major tile into feature-major
            dst[:, f, t0:t0+P] for f in 0..5."""
            if DMA_TRANSPOSE:
                dma_eng.dma_start_transpose(out=dst[:, :, t0:t0 + P], in_=lt)
                return
            pst = ps_pool.tile([P, NC6, P], BF16, name="ps_tr")
            for f in range(NC6):
                nc.tensor.transpose(out=pst[:, f, :],
                                    in_=lt[:, f * P:(f + 1) * P],
                                    identity=ident)
            nc.vector.tensor_copy(out=dst[:, :, t0:t0 + P], in_=pst)

        with ExitStack() as esB:
            pB = esB.enter_context(tc.tile_pool(name="pB", bufs=1))
            k_fm = pB.tile([P, NC6, T], BF16, name="k_fm")
            v_st = pB.tile([P, NT, H, HD + 1], BF16, name="v_st")
            q_fm = pB.tile([P, NC6, TOK], BF16, name="q_fm")
            y_acc = pB.tile([P, H, TOK], BF16, name="y_acc")
            masks_sb = pB.tile([P, NT, P], BF16, name="masks_sb")
            nc.vector.memset(v_st[:, :, :, HD:HD + 1], 1.0)

            with ExitStack() as esA:
                pA = esA.enter_context(tc.tile_pool(name="pA", bufs=1))
                psA = esA.enter_context(
                    tc.tile_pool(name="psA", bufs=BUFS_PSA, space="PSUM"))
                psS = esA.enter_context(
                    tc.tile_pool(name="psS", bufs=BUFS_PSS, space="PSUM"))
                psY = esA.enter_context(
                    tc.tile_pool(name="psY", bufs=BUFS_PSY, space="PSUM"))
                pexp = esA.enter_context(
                    tc.tile_pool(name="pexp", bufs=BUFS_PEXP))
                pLN = esA.enter_context(tc.tile_pool(name="pLN", bufs=2))
                pXC = esA.enter_context(tc.tile_pool(name="pXC", bufs=2))
                ln1q_fm = pA.tile([P, NC6, TOK], BF16, name="ln1q_fm")
                wqkv_sb = pA.tile([P, 3, NC6, C], BF16, name="wqkv_sb")

                # ---- activation loads on the sync queue, in need order
                # (halved so LN stats can start after the first two tiles)
                xq4 = xq_d[:, :].rearrange("(j p) c -> p j c", p=P)
                nc.sync.dma_start(out=xq_sb[:, 0:2], in_=xq4[:, 0:2])
                nc.sync.dma_start(out=xq_sb[:, 2:4], in_=xq4[:, 2:4])
                # ---- weights/masks on the gpsimd (Pool) queue, in need
                # order, so they never block ACT compute or activation loads
                wq3 = wqkv_d[:, :].rearrange("p (s ci c) -> p s ci c", s=3, ci=NC6)
                nc.gpsimd.dma_start(out=wqkv_sb[:, 0], in_=wq3[:, 0])
                nc.gpsimd.dma_start(out=wqkv_sb[:, 1], in_=wq3[:, 1])
                nc.gpsimd.dma_start(out=wqkv_sb[:, 2], in_=wq3[:, 2])
                nc.gpsimd.dma_start(
                    out=masks_sb,
                    in_=masks_d[:, :].rearrange("p (c m) -> p c m", c=NT))
                nc.gpsimd.dma_start(
                    out=wo_sb,
                    in_=wo_d[:, :].rearrange("p (ci c) -> p ci c", ci=NC6))

                # -------- phase 1a: LN1 over own q tiles, then Q projection
                def emit_q(t, lt):
                    transpose_fm(ln1q_fm, t * P, lt, psA, nc.scalar)
                ln_chunk(xq_sb[:, 0:2], 2, emit_q, act_norm=True)

                def emit_q2(t, lt):
                    transpose_fm(ln1q_fm, (t + 2) * P, lt, psA, nc.scalar)
                ln_chunk(xq_sb[:, 2:4], 2, emit_q2, act_norm=True)

                for f in range(NC6):
                    ps = psA.tile([P, 512], F32, name="ps_k")
                    for ci in range(NC6):
                        nc.tensor.matmul(
                            ps, lhsT=wqkv_sb[:, 0, ci, f * P:(f + 1) * P],
                            rhs=ln1q_fm[:, ci, :],
                            start=(ci == 0), stop=(ci == NC6 - 1))
                    nc.vector.tensor_scalar_add(out=q_fm[:, f, :], in0=ps,
                                                scalar1=qkvb[:, f:f + 1])

                # -------- phases 1b/2/3 interleaved per 512-token chunk:
                # LN1(chunk) -> K(chunk), V(chunk) -> attention(k-group=chunk)
                for n in range(4):
                    xc = pXC.tile([P, 4, C], F32, name="ph1_xc")
                    nc.sync.dma_start(
                        out=xc,
                        in_=x_ctx_d[n * 512:(n + 1) * 512, :].rearrange(
                            "(t p) c -> p t c", p=P))

                    ln_c = pLN.tile([P, NC6, 512], BF16, name="ln_c")

                    def emit_ctx(t, lt, ln_c=ln_c):
                        transpose_fm(ln_c, t * P, lt, psA, nc.scalar)
                    ln_chunk(xc, 4, emit_ctx)

                    # K projection for this chunk (feature-major)
                    for f in range(NC6):
                        ps = psA.tile([P, 512], F32, name="ps_k")
                        for ci in range(NC6):
                            nc.tensor.matmul(
                                ps,
                                lhsT=wqkv_sb[:, 1, ci, f * P:(f + 1) * P],
                                rhs=ln_c[:, ci, :],
                                start=(ci == 0), stop=(ci == NC6 - 1))
                        nc.vector.tensor_scalar_add(
                            out=k_fm[:, f, n * 512:(n + 1) * 512], in0=ps,
                            scalar1=qkvb[:, NC6 + f:NC6 + f + 1])
                    # V projection for this chunk (token-major, +ones row)
                    for t in range(n * 4, n * 4 + 4):
                        for half in range(2):
                            ps = psA.tile([P, 512], F32, name="ps_k")
                            for ci in range(NC6):
                                nc.tensor.matmul(
                                    ps[:, 0:384],
                                    lhsT=ln_c[:, ci, (t - n * 4) * P:
                                              (t - n * 4 + 1) * P],
                                    rhs=wqkv_sb[:, 2, ci,
                                                half * 384:(half + 1) * 384],
                                    start=(ci == 0), stop=(ci == NC6 - 1))
                            nc.vector.tensor_add(
                                out=v_st[:, t, half * 6:(half + 1) * 6, 0:HD],
                                in0=ps[:, 0:384].rearrange(
                                    "p (h d) -> p h d", h=6),
                                in1=vb_bc[:, half * 384:(half + 1) * 384].rearrange(
                                    "p (h d) -> p h d", h=6))

                    # ---- attention for k-group n against q-slots n..3
                    g4 = n
                    ncols = TOK - g4 * P
                    qoff = g4 * P
                    for hp in range(H // 2):
                        ps_y = [psY.tile([P, 512], F32, name="ps_y")
                                for _ in range(2)]
                        for c in range(g4 * 4, g4 * 4 + 4):
                            ss = psS.tile([P, 2, 512], F32, name="ps_s")
                            for i in range(2):
                                lo = 64 * i
                                nc.tensor.matmul(
                                    ss[:, i, 0:ncols],
                                    lhsT=k_fm[lo:lo + 64, hp, c * P:(c + 1) * P],
                                    rhs=q_fm[lo:lo + 64, hp, qoff:TOK],
                                    start=True, stop=True,
                                    tile_position=(lo, 0))
                            pe = pexp.tile([P, 2, 512], BF16, name="pe")
                            nc.scalar.activation(
                                out=pe[:, :, 0:ncols], in_=ss[:, :, 0:ncols],
                                func=AF.Exp, scale=0.125)
                            if MASK_BCAST:
                                nc.vector.tensor_mul(
                                    out=pe[:, :, 0:P], in0=pe[:, :, 0:P],
                                    in1=masks_sb[:, c:c + 1, :].to_broadcast(
                                        (P, 2, P)))
                            else:
                                for i in range(2):
                                    nc.vector.tensor_mul(
                                        out=pe[:, i, 0:P], in0=pe[:, i, 0:P],
                                        in1=masks_sb[:, c, :])
                            for i in range(2):
                                nc.tensor.matmul(
                                    ps_y[i][0:HD + 1, 0:ncols],
                                    lhsT=v_st[:, c, 2 * hp + i, :],
                                    rhs=pe[:, i, 0:ncols],
                                    start=(c == g4 * 4), stop=(c == g4 * 4 + 3))
                        for i in range(2):
                            h = 2 * hp + i
                            if g4 == 0:
                                nc.vector.tensor_copy(
                                    out=y_acc[0:HD + 1, h, qoff:TOK],
                                    in_=ps_y[i][0:HD + 1, 0:ncols])
                            else:
                                nc.vector.tensor_add(
                                    out=y_acc[0:HD + 1, h, qoff:TOK],
                                    in0=y_acc[0:HD + 1, h, qoff:TOK],
                                    in1=ps_y[i][0:HD + 1, 0:ncols])
                        if g4 != 3:
                            continue
                        # ---- softmax renormalization for this head pair:
                        # y = y_acc[0:64] / y_acc[64], emitted inside the
                        # last chunk so y_acc is consumed as early as possible
                        for i in range(2):
                            h = 2 * hp + i
                            lo = 64 * i
                            dn = small.tile([1, TOK], F32R, name="dn")
                            with nc.allow_low_precision(
                                    reason="fp32r rounding of softmax recips"):
                                nc.vector.reciprocal(out=dn,
                                                     in_=y_acc[HD:HD + 1, h, :])
                            # broadcast recip row to 64 partitions via K=1
                            # outer product on the PE (fp32r at full rate)
                            rb_ps = psY.tile([P, 512], F32, name="ps_y")
                            nc.tensor.matmul(rb_ps[0:64, :], lhsT=ones64[:, :],
                                             rhs=dn[:, :], start=True, stop=True)
                            rb = work.tile([P, TOK], F32, name="rb")
                            nc.scalar.copy(out=rb[0:64, :], in_=rb_ps[0:64, :])
                            nc.vector.tensor_mul(out=y_fm[lo:lo + 64, hp, :],
                                                 in0=y_acc[0:HD, h, :],
                                                 in1=rb[0:64, :])

        # ---------------- phase 4: w_o projection + residual + LN2
        with ExitStack() as es45:
            pD = es45.enter_context(tc.tile_pool(name="pD", bufs=1))
            ps45 = es45.enter_context(tc.tile_pool(name="ps45", bufs=6,
                                                   space="PSUM"))
            psT2 = es45.enter_context(tc.tile_pool(name="psT2", bufs=2,
                                                   space="PSUM"))
            x2_sb = pD.tile([P, GQ, C], F32, name="x2_sb")
            ln2_fm = pD.tile([P, NC6, TOK], BF16, name="ln2_fm")
            wfc_sb = pD.tile([P, NC6, DFF], BF16, name="wfc_sb")
            wproj_sb = pD.tile([P, ND, C], BF16, name="wproj_sb")
            h_fm = pD.tile([P, ND, TOK], BF16, name="h_fm")
            # quarter-split weight loads so MLP matmuls can start as soon as
            # the first slice lands (the loads are WAR-gated on attention
            # buffers being released)
            wfc3 = wfc_d[:, :].rearrange("p (ci f) -> p ci f", ci=NC6)
            for qtr in range(4):
                nc.gpsimd.dma_start(
                    out=wfc_sb[:, :, qtr * DFF // 4:(qtr + 1) * DFF // 4],
                    in_=wfc3[:, :, qtr * DFF // 4:(qtr + 1) * DFF // 4])
            wp3 = wproj_d[:, :].rearrange("p (di c) -> p di c", di=ND)
            for qtr in range(4):
                nc.gpsimd.dma_start(
                    out=wproj_sb[:, qtr * ND // 4:(qtr + 1) * ND // 4],
                    in_=wp3[:, qtr * ND // 4:(qtr + 1) * ND // 4])

            for j in range(GQ):
                for half in range(2):
                    ps = ps45.tile([P, 512], F32, name="ps_mm")
                    for ci in range(NC6):
                        nc.tensor.matmul(
                            ps[:, 0:384],
                            lhsT=y_fm[:, ci, j * P:(j + 1) * P],
                            rhs=wo_sb[:, ci, half * 384:(half + 1) * 384],
                            start=(ci == 0), stop=(ci == NC6 - 1))
                    nc.vector.tensor_add(
                        out=x2_sb[:, j, half * 384:(half + 1) * 384],
                        in0=ps[:, 0:384],
                        in1=xq_sb[:, j, half * 384:(half + 1) * 384])

                def emit_ln2(t, lt, j=j):
                    transpose_fm(ln2_fm, j * P, lt, psT2, nc.sync)
                ln_chunk(x2_sb[:, j:j + 1], 1, emit_ln2, act_norm=True)

            # ---------------- phase 5: MLP
            for d in range(ND):
                ps = ps45.tile([P, 512], F32, name="ps_mm")
                for ci in range(NC6):
                    nc.tensor.matmul(
                        ps, lhsT=wfc_sb[:, ci, d * P:(d + 1) * P],
                        rhs=ln2_fm[:, ci, :],
                        start=(ci == 0), stop=(ci == NC6 - 1))
                nc.scalar.activation(out=h_fm[:, d, :], in_=ps, func=AF.Gelu,
                                     bias=fcb[:, d:d + 1])
            for j in range(GQ):
                ob = work.tile([P, C], F32, name="ph5_ob")
                for half in range(2):
                    ps = ps45.tile([P, 512], F32, name="ps_mm")
                    for di in range(ND):
                        nc.tensor.matmul(
                            ps[:, 0:384],
                            lhsT=h_fm[:, di, j * P:(j + 1) * P],
                            rhs=wproj_sb[:, di, half * 384:(half + 1) * 384],
                            start=(di == 0), stop=(di == ND - 1))
                    nc.vector.tensor_add(out=ob[:, half * 384:(half + 1) * 384],
                                         in0=ps[:, 0:384],
                                         in1=x2_sb[:, j, half * 384:(half + 1) * 384])
                nc.vector.tensor_add(out=ob, in0=ob, in1=bproj_bc)
                nc.sync.dma_start(out=out_d[j * P:(j + 1) * P, :], in_=ob)


# ------------------------------------------------------------- host wrapper
_NC_CACHE = {}


def _get_nc(nreps: int = 1):
    if nreps not in _NC_CACHE:
        _NC_CACHE[nreps] = build_program(nreps)
    return _NC_CACHE[nreps]


def make_in_maps(x, ln1_g, ln1_b, w_qkv, w_o, ln2_g, ln2_b, w_fc, b_fc,
                 w_proj, b_proj):
    """Host-side sharding: returns list of 8 per-core input dicts."""
    x = np.asarray(x, np.float32)
    ln1_g = np.asarray(ln1_g, np.float64)
    ln1_b = np.asarray(ln1_b, np.float64)
    ln2_g = np.asarray(ln2_g, np.float64)
    ln2_b = np.asarray(ln2_b, np.float64)
    w_qkv64 = np.asarray(w_qkv, np.float64)
    w_fc64 = np.asarray(w_fc, np.float64)

    # fold LN gains into the following weights; LN betas into their biases
    w_qkv_eff = (ln1_g[:, None] * w_qkv64)
    qkv_bias = ln1_b @ w_qkv64
    w_fc_eff = (ln2_g[:, None] * w_fc64)
    fc_bias = np.asarray(b_fc, np.float64) + ln2_b @ w_fc64

    wqkv_bf = w_qkv_eff.astype(np.float32).astype(ml_dtypes.bfloat16)
    wo_bf = np.asarray(w_o, np.float32).astype(ml_dtypes.bfloat16)
    wfc_bf = w_fc_eff.astype(np.float32).astype(ml_dtypes.bfloat16)
    wproj_bf = np.asarray(w_proj, np.float32).astype(ml_dtypes.bfloat16)

    # pre-swizzle weights into SBUF layout (contiguous single-DMA transfers)
    # w_qkv [C, 3C] -> [P, 3(qkv), NC6, C]: element [p, s, ci, f] = W[ci*P+p, s*C+f]
    wqkv_sw = np.ascontiguousarray(
        wqkv_bf.reshape(NC6, P, 3, C).transpose(1, 2, 0, 3).reshape(P, -1))
    wo_sw = np.ascontiguousarray(
        wo_bf.reshape(NC6, P, C).transpose(1, 0, 2).reshape(P, -1))
    wfc_sw = np.ascontiguousarray(
        wfc_bf.reshape(NC6, P, DFF).transpose(1, 0, 2).reshape(P, -1))
    wproj_sw = np.ascontiguousarray(
        wproj_bf.reshape(ND, P, C).transpose(1, 0, 2).reshape(P, -1))

    qkvb_t = np.ascontiguousarray(
        qkv_bias.astype(np.float32).reshape(3 * NC6, P).T)
    fcb_t = np.ascontiguousarray(fc_bias.astype(np.float32).reshape(ND, P).T)
    vb_row = np.ascontiguousarray(qkv_bias[2 * C:].astype(np.float32)[None, :])
    bproj_row = np.ascontiguousarray(
        np.asarray(b_proj, np.float32)[None, :])

    in_maps = []
    for b in range(B):
        for g in range(4):
            qt = QTILES[g]
            xq = np.concatenate([x[b, p * P:(p + 1) * P] for p in qt], axis=0)
            masks = np.zeros((NT, P, P), np.float32)
            for c in range(NT):
                pos = qt[c // 4]
                kk = c * P + np.arange(P)[:, None]
                qq = pos * P + np.arange(P)[None, :]
                masks[c] = (kk <= qq).astype(np.float32)
            masks_sw = np.ascontiguousarray(
                masks.astype(ml_dtypes.bfloat16).transpose(1, 0, 2).reshape(P, -1))
            in_maps.append({
                "x_ctx": np.ascontiguousarray(x[b]),
                "xq": np.ascontiguousarray(xq),
                "ident": np.eye(P, dtype=ml_dtypes.bfloat16),
                "ones_row": np.ones((1, 64), np.float32),
                "w_qkv": wqkv_sw,
                "qkv_b": qkvb_t,
                "vb_row": vb_row,
                "w_o": wo_sw,
                "w_fc": wfc_sw,
                "fc_b": fcb_t,
                "w_proj": wproj_sw,
                "bproj_row": bproj_row,
                "masks": masks_sw,
            })
    return in_maps


def assemble_output(results):
    out = np.empty((B, T, C), np.float32)
    for b in range(B):
        for g in range(4):
            r = results[b * 4 + g]["out_q"]
            for j, p in enumerate(QTILES[g]):
                out[b, p * P:(p + 1) * P] = r[j * P:(j + 1) * P]
    return out


def kernel(**inputs) -> np.ndarray:
    nc = _get_nc(1)
    in_maps = make_in_maps(**inputs)
    res = run_bass_kernel_spmd(nc, in_maps, core_ids=list(range(8)))
    return assemble_output(res.results)
